# revision 31
# baseline (speedup 1.0000x reference)
"""Trainium2 Bass kernel for masked attention (nn_Attention1).

Math (per batch b):
    q = query @ Wq_w.T + Wq_b        # [L, D]
    k = key   @ Wk_w.T + Wk_b        # [L, D]
    S = q @ k.T / sqrt(D)            # [L, L]
    S = where(mask==0, -1e9, S)      # mask over key positions
    out = softmax(S, -1) @ value     # [L, D]

Strategy:
  - Batch (B=8) sharded across the 8 NeuronCores, weights replicated.
  - mask==0 keys contribute exactly 0 after softmax (exp underflows to 0
    in fp32), so each core gathers only its unmasked K/V rows on-device
    (dma_gather with host-computed indices) and runs dense attention over
    the compacted Lk. Padding rows get an additive -1e9 bias -> exp = 0.
  - Scores are computed transposed (S^T layout, lk on partitions) so the
    softmax mask is a per-partition ACT bias and the PV matmul needs no
    P transpose. The softmax denominator rides along as a ones-column
    appended to V (column 256 of the PV accumulation).
  - No max-subtraction: |S| <= ~10 for this problem scale, exp is safe.
  - 1/sqrt(D) is folded into Wq/bq on the host.
  - Matmuls run as float32r (full-rate fp32 mode); PE transposes stay
    plain float32 (exact).
"""

import math
import sys

for _p in ("/root/.axon_site", "/root/.axon_site/_ro/trn_rl_repo",
           "/root/.axon_site/_ro/pypackages", "/opt/trn_rl_repo"):
    if _p not in sys.path:
        sys.path.append(_p)

import numpy as np

import concourse.bass as bass
import concourse.tile as tile
from concourse import mybir
from concourse.bass_utils import run_bass_kernel_spmd

P = 128
B, L, D = 8, 2048, 256
F32 = mybir.dt.float32
F32R = mybir.dt.float32r
NEG = -1e9

_MAX_WAITS = 1


def _split_excess_waits(nc):
    """walrus rejects instructions with more than one sem wait; split extra
    waits onto preceding same-engine NOPs (engines execute in order, so the
    semantics are identical)."""
    for f in nc.m.functions:
        for blk in f.blocks:
            insts = blk.instructions
            i = 0
            while i < len(insts):
                inst = insts[i]
                si = inst.sync_info
                if si is not None and si.on_wait and len(si.on_wait) > _MAX_WAITS:
                    waits = list(si.on_wait)
                    chunks = [waits[j:j + _MAX_WAITS]
                              for j in range(0, len(waits), _MAX_WAITS)]
                    *nop_chunks, last = chunks
                    nops = []
                    for k, ch in enumerate(nop_chunks):
                        nop = mybir.InstNoOp(
                            name=f"{inst.name}-waitsplit{k}", ins=[], outs=[],
                            sync_info=mybir.SyncInfo(on_wait=ch, on_update=[]),
                        )
                        nop.engine = inst.engine
                        nops.append(nop)
                    inst.sync_info = mybir.SyncInfo(
                        on_wait=last, on_update=list(si.on_update or []))
                    insts[i:i] = nops
                    i += len(nops)
                i += 1


def _build(lk, mm_dt=F32R):
    """Build the single-core program; every core runs it on its own batch.

    lk: padded count of gathered key rows (multiple of 128).
    """
    t_lk = lk // P
    t_lq = L // P
    lq_blk = 1024 if t_lk <= 12 else 512
    n_blk = L // lq_blk
    idx_cols = lk // 16

    nc = bass.Bass("TRN2", target_bir_lowering=False, debug=False,
                   num_devices=8)

    q_in = nc.dram_tensor("q_in", [L, D], F32, kind="ExternalInput").ap()
    k_in = nc.dram_tensor("k_in", [L, D], F32, kind="ExternalInput").ap()
    v_in = nc.dram_tensor("v_in", [L, D], F32, kind="ExternalInput").ap()
    idx_in = nc.dram_tensor("idx_in", [lk], mybir.dt.int32,
                            kind="ExternalInput").ap()
    id_in = nc.dram_tensor("id_in", [P, P], mm_dt,
                           kind="ExternalInput").ap()
    mb_in = nc.dram_tensor("mb_in", [lk], F32, kind="ExternalInput").ap()
    wq_in = nc.dram_tensor("wq_in", [D, D], F32, kind="ExternalInput").ap()
    wk_in = nc.dram_tensor("wk_in", [D, D], F32, kind="ExternalInput").ap()
    bq_in = nc.dram_tensor("bq_in", [D], F32, kind="ExternalInput").ap()
    bk_in = nc.dram_tensor("bk_in", [D], F32, kind="ExternalInput").ap()
    out = nc.dram_tensor("out", [L, D], F32, kind="ExternalOutput").ap()

    with tile.TileContext(nc) as tc:
        with (
            tc.tile_pool(name="consts", bufs=1) as consts,
            tc.tile_pool(name="stage", bufs=1) as stage,
            tc.tile_pool(name="acts", bufs=1) as acts,
            tc.tile_pool(name="pt", bufs=t_lk + 3) as ptp,
            tc.tile_pool(name="eplg", bufs=4) as eplg,
            tc.tile_pool(name="pp_ps", bufs=2, space="PSUM") as pp_ps,
            tc.tile_pool(name="s_ps", bufs=3, space="PSUM") as s_ps,
        ):
            # ---- constants; gather indices first (gathers gate the k path)
            idx_sb = consts.tile([P, t_lk], mybir.dt.int32)
            nc.sync.dma_start(out=idx_sb, in_=idx_in.rearrange("(t p) -> p t", p=P))
            ident = consts.tile([P, P], mm_dt)
            nc.scalar.dma_start(out=ident, in_=id_in)

            # ---- load query; gather key/value rows -------------------------
            q_stage = stage.tile([P, t_lq, D], mm_dt)
            q_view = q_in.rearrange("(t p) d -> p t d", p=P).bitcast(mm_dt)
            for c in range(0, t_lq, 4):
                nc.sync.dma_start(out=q_stage[:, c:c + 4, :],
                                  in_=q_view[:, c:c + 4, :])

            wq_sb = consts.tile([P, 2, D], mm_dt)
            nc.scalar.dma_start(out=wq_sb,
                              in_=wq_in.rearrange("(a p) d -> p a d", p=P).bitcast(mm_dt))
            wk_sb = consts.tile([P, 2, D], mm_dt)
            nc.scalar.dma_start(out=wk_sb,
                              in_=wk_in.rearrange("(a p) d -> p a d", p=P).bitcast(mm_dt))
            bq_sb = consts.tile([P, 2], F32)
            nc.scalar.dma_start(out=bq_sb, in_=bq_in.rearrange("(h p) -> p h", p=P))
            bk_sb = consts.tile([P, 2], F32)
            nc.scalar.dma_start(out=bk_sb, in_=bk_in.rearrange("(h p) -> p h", p=P))
            mb_sb = consts.tile([P, t_lk], F32)
            nc.scalar.dma_start(out=mb_sb, in_=mb_in.rearrange("(t p) -> p t", p=P))

            k_stage = stage.tile([P, t_lk, D], mm_dt)
            v_stage = stage.tile([P, t_lk, D], F32)
            for srcd, dst in ((k_in.bitcast(mm_dt), k_stage),
                              (v_in, v_stage)):
                for t in range(t_lk):
                    nc.gpsimd.indirect_dma_start(
                        out=dst[:, t, :],
                        out_offset=None,
                        in_=srcd,
                        in_offset=bass.IndirectOffsetOnAxis(
                            ap=idx_sb[:, t:t + 1], axis=0),
                    )

            # ---- PE warm-up: HAM un-throttles (1.2 -> 2.4 GHz) only after
            # ~3.4us of sustained activity, and re-throttles after ~3.4us
            # idle. Tiny matmuls on a memset scratch (no DMA dependency)
            # bridge the DMA front so the real work starts at full clock.
            warm_sb = consts.tile([P, 16], mm_dt)
            nc.vector.memset(warm_sb.bitcast(F32), 1.0)
            for _ in range(32):
                wp = pp_ps.tile([16, 16], F32, tag="pp")
                nc.tensor.matmul(wp, lhsT=warm_sb, rhs=warm_sb,
                                 start=True, stop=True)

            # ---- transpose + project, interleaved per 512-row chunk so the
            # PE follows data arrival (q first, then gathered k) ------------
            q_tr = acts.tile([P, 2, L], mm_dt)
            k_tr = acts.tile([P, 2, lk], mm_dt)
            q_t = acts.tile([P, 2, L], mm_dt)
            k_t = acts.tile([P, 2, lk], mm_dt)
            q_path = (q_stage, q_tr, wq_sb, bq_sb, q_t, t_lq, L)
            k_path = (k_stage, k_tr, wk_sb, bk_sb, k_t, t_lk, lk)

            def emit_chunk(path, c0):
                srcd, dst, w_sb, b_sb, x_t, nt, ln = path
                cw = min(512, ln - c0)
                for t in range(c0 // P, min(nt, (c0 + 512) // P)):
                    for h in range(2):
                        tp = s_ps.tile([P, P], mm_dt, tag="sp")
                        nc.tensor.transpose(
                            out=tp,
                            in_=srcd[:, t, h * P:(h + 1) * P],
                            identity=ident)
                        nc.vector.tensor_copy(
                            out=dst[:, h, t * P:(t + 1) * P], in_=tp)
                for h in range(2):
                    pp = pp_ps.tile([P, 512], F32, tag="pp")
                    for a in range(2):
                        nc.tensor.matmul(
                            pp[:, :cw],
                            lhsT=w_sb[:, a, h * P:(h + 1) * P],
                            rhs=dst[:, a, c0:c0 + cw],
                            start=(a == 0), stop=(a == 1))
                    nc.vector.tensor_scalar_add(
                        out=x_t[:, h, c0:c0 + cw], in0=pp[:, :cw],
                        scalar1=b_sb[:, h:h + 1])

            # interleave q/k chunks by expected data arrival: q is on-chip
            # early, gathered k tiles trickle in from the serial Q7 gathers
            order = [(q_path, 0), (q_path, 512), (k_path, 0), (q_path, 1024),
                     (k_path, 512), (q_path, 1536)]
            order += [(k_path, c0) for c0 in range(1024, lk, 512)]
            for path, c0 in order:
                emit_chunk(path, c0)

            # V with a ones-column appended: PV accumulates the softmax
            # denominator into column 256. Width D+2 keeps the free size
            # even and offsets 8B-aligned (fp32r matmul ISA restriction).
            # Built here (not earlier): only needed once PV starts.
            v_ext = acts.tile([P, t_lk, D + 2], mm_dt)
            for t0 in range(0, t_lk, 3):
                t1 = min(t0 + 3, t_lk)
                nc.gpsimd.tensor_copy(out=v_ext[:, t0:t1, 0:D],
                                      in_=v_stage[:, t0:t1, :])
            # memset can't encode an fp32r value type; 1.0f is bit-identical
            nc.gpsimd.memset(v_ext[:, :, D:D + 2].bitcast(F32), 1.0)

            # ---- attention over lq blocks ----------------------------------
            for blk in range(n_blk):
                b0 = blk * lq_blk
                pts = []
                for t in range(t_lk):
                    sp = s_ps.tile([P, lq_blk], F32, tag="sp")
                    for h in range(2):
                        for c0 in range(0, lq_blk, 512):
                            nc.tensor.matmul(
                                sp[:, c0:c0 + 512],
                                lhsT=k_t[:, h, t * P:(t + 1) * P],
                                rhs=q_t[:, h, b0 + c0:b0 + c0 + 512],
                                start=(h == 0), stop=(h == 1))
                    pt = ptp.tile([P, lq_blk], mm_dt, tag="pt")
                    nc.scalar.activation(out=pt, in_=sp,
                                         func=mybir.ActivationFunctionType.Exp,
                                         bias=mb_sb[:, t:t + 1], scale=1.0)
                    pts.append(pt)

                for qt in range(lq_blk // P):
                    op = pp_ps.tile([P, D + 2], F32, tag="pp")
                    for t in range(t_lk):
                        nc.tensor.matmul(
                            op,
                            lhsT=pts[t][:, qt * P:(qt + 1) * P],
                            rhs=v_ext[:, t, :],
                            start=(t == 0), stop=(t == t_lk - 1))
                    rcp = eplg.tile([P, 1], F32, tag="rcp")
                    nc.vector.reciprocal(out=rcp, in_=op[:, D:D + 1])
                    o_sb = eplg.tile([P, D], F32, tag="osb")
                    nc.vector.tensor_scalar_mul(out=o_sb, in0=op[:, 0:D],
                                                scalar1=rcp)
                    r0 = b0 + qt * P
                    nc.scalar.dma_start(out=out[r0:r0 + P, :], in_=o_sb)

    return nc


_PROG_CACHE = {}


def _get_program(lk, mm_dt=F32R):
    key = (lk, str(mm_dt))
    if key not in _PROG_CACHE:
        nc = _build(lk, mm_dt)
        # populate .instr bytes for InstISA subclasses (the library reload);
        # raw Bass skips the Bacc pass that does this
        mybir.codegen_inst_isa_subclasses(nc)
        _split_excess_waits(nc)  # only needed for walrus codegen (HW path)
        _PROG_CACHE[key] = nc
    return _PROG_CACHE[key]


def _prep_inputs(query, key, value, mask, Wq_w, Wq_b, Wk_w, Wk_b):
    """Host-side prep: fold 1/sqrt(D) into Wq, transpose weights, build
    per-batch gather indices + additive mask bias for padding."""
    scale = 1.0 / math.sqrt(D)
    wq_t = np.ascontiguousarray((Wq_w * scale).T, dtype=np.float32)
    wk_t = np.ascontiguousarray(Wk_w.T, dtype=np.float32)
    bq = (Wq_b * scale).astype(np.float32)
    bk = Wk_b.astype(np.float32)

    idxs = [np.nonzero(mask[b])[0] for b in range(B)]
    counts = [len(ix) for ix in idxs]
    lk = max(P, -(-max(counts) // P) * P)  # round up to multiple of 128

    in_maps = []
    for b in range(B):
        n = counts[b]
        idx = np.zeros(lk, dtype=np.int32)
        idx[:n] = idxs[b]
        mb = np.zeros(lk, dtype=np.float32)
        mb[n:] = NEG
        in_maps.append({
            "q_in": np.ascontiguousarray(query[b], dtype=np.float32),
            "k_in": np.ascontiguousarray(key[b], dtype=np.float32),
            "v_in": np.ascontiguousarray(value[b], dtype=np.float32),
            "idx_in": idx,
            "id_in": np.eye(P, dtype=np.float32),
            "mb_in": mb,
            "wq_in": wq_t, "wk_in": wk_t, "bq_in": bq, "bk_in": bk,
        })
    return in_maps, lk, counts


def _reference_batch_np(query, key, value, mask, Wq_w, Wq_b, Wk_w, Wk_b):
    """Exact numpy replica of the reference for degenerate batches
    (a batch whose mask is all zeros -> uniform softmax)."""
    q = query @ Wq_w.T + Wq_b
    k = key @ Wk_w.T + Wk_b
    s = (q @ k.T) / math.sqrt(D)
    m = mask.astype(s.dtype)[None, :]
    s = np.where(m == 0, np.float32(NEG), s * m)
    s = s - s.max(-1, keepdims=True)
    e = np.exp(s)
    attn = e / e.sum(-1, keepdims=True)
    return (attn @ value).astype(np.float32)


def _run(inputs, mm_dt=F32R, trace=False):
    query = np.asarray(inputs["query"], dtype=np.float32)
    key = np.asarray(inputs["key"], dtype=np.float32)
    value = np.asarray(inputs["value"], dtype=np.float32)
    mask = np.asarray(inputs["mask"])
    Wq_w = np.asarray(inputs["Wq_w"], dtype=np.float32)
    Wq_b = np.asarray(inputs["Wq_b"], dtype=np.float32)
    Wk_w = np.asarray(inputs["Wk_w"], dtype=np.float32)
    Wk_b = np.asarray(inputs["Wk_b"], dtype=np.float32)

    in_maps, lk, counts = _prep_inputs(query, key, value, mask,
                                       Wq_w, Wq_b, Wk_w, Wk_b)
    nc = _get_program(lk, mm_dt)
    res = run_bass_kernel_spmd(nc, in_maps, core_ids=list(range(B)),
                               trace=trace)
    out = np.stack([res.results[b]["out"] for b in range(B)])

    for b in range(B):
        if counts[b] == 0:  # degenerate: softmax over all -1e9 is uniform
            out[b] = _reference_batch_np(query[b], key[b], value[b], mask[b],
                                         Wq_w, Wq_b, Wk_w, Wk_b)
    return out, res


def kernel(**inputs) -> np.ndarray:
    out, _ = _run(inputs)
    return out


# revision 32
# speedup vs baseline: 1.0191x; 1.0191x over previous
"""Trainium2 Bass kernel for masked attention (nn_Attention1).

Math (per batch b):
    q = query @ Wq_w.T + Wq_b        # [L, D]
    k = key   @ Wk_w.T + Wk_b        # [L, D]
    S = q @ k.T / sqrt(D)            # [L, L]
    S = where(mask==0, -1e9, S)      # mask over key positions
    out = softmax(S, -1) @ value     # [L, D]

Strategy:
  - Batch (B=8) sharded across the 8 NeuronCores, weights replicated.
  - mask==0 keys contribute exactly 0 after softmax (exp underflows to 0
    in fp32), so each core gathers only its unmasked K/V rows on-device
    (dma_gather with host-computed indices) and runs dense attention over
    the compacted Lk. Padding rows get an additive -1e9 bias -> exp = 0.
  - Scores are computed transposed (S^T layout, lk on partitions) so the
    softmax mask is a per-partition ACT bias and the PV matmul needs no
    P transpose. The softmax denominator rides along as a ones-column
    appended to V (column 256 of the PV accumulation).
  - No max-subtraction: |S| <= ~10 for this problem scale, exp is safe.
  - 1/sqrt(D) is folded into Wq/bq on the host.
  - Matmuls run as float32r (full-rate fp32 mode); PE transposes stay
    plain float32 (exact).
"""

import math
import sys

for _p in ("/root/.axon_site", "/root/.axon_site/_ro/trn_rl_repo",
           "/root/.axon_site/_ro/pypackages", "/opt/trn_rl_repo"):
    if _p not in sys.path:
        sys.path.append(_p)

import numpy as np

import concourse.bass as bass
import concourse.tile as tile
from concourse import mybir
from concourse.bass_utils import run_bass_kernel_spmd

P = 128
B, L, D = 8, 2048, 256
F32 = mybir.dt.float32
F32R = mybir.dt.float32r
NEG = -1e9

_MAX_WAITS = 1


def _split_excess_waits(nc):
    """walrus rejects instructions with more than one sem wait; split extra
    waits onto preceding same-engine NOPs (engines execute in order, so the
    semantics are identical)."""
    for f in nc.m.functions:
        for blk in f.blocks:
            insts = blk.instructions
            i = 0
            while i < len(insts):
                inst = insts[i]
                si = inst.sync_info
                if si is not None and si.on_wait and len(si.on_wait) > _MAX_WAITS:
                    waits = list(si.on_wait)
                    chunks = [waits[j:j + _MAX_WAITS]
                              for j in range(0, len(waits), _MAX_WAITS)]
                    *nop_chunks, last = chunks
                    nops = []
                    for k, ch in enumerate(nop_chunks):
                        nop = mybir.InstNoOp(
                            name=f"{inst.name}-waitsplit{k}", ins=[], outs=[],
                            sync_info=mybir.SyncInfo(on_wait=ch, on_update=[]),
                        )
                        nop.engine = inst.engine
                        nops.append(nop)
                    inst.sync_info = mybir.SyncInfo(
                        on_wait=last, on_update=list(si.on_update or []))
                    insts[i:i] = nops
                    i += len(nops)
                i += 1


def _build(lk, mm_dt=F32R):
    """Build the single-core program; every core runs it on its own batch.

    lk: padded count of gathered key rows (multiple of 128).
    """
    t_lk = lk // P
    t_lq = L // P
    lq_blk = 1024 if t_lk <= 12 else 512
    n_blk = L // lq_blk
    idx_cols = lk // 16

    nc = bass.Bass("TRN2", target_bir_lowering=False, debug=False,
                   num_devices=8)

    q_in = nc.dram_tensor("q_in", [L, D], F32, kind="ExternalInput").ap()
    k_in = nc.dram_tensor("k_in", [L, D], F32, kind="ExternalInput").ap()
    v_in = nc.dram_tensor("v_in", [L, D], F32, kind="ExternalInput").ap()
    idx_in = nc.dram_tensor("idx_in", [lk], mybir.dt.int32,
                            kind="ExternalInput").ap()
    id_in = nc.dram_tensor("id_in", [P, P], mm_dt,
                           kind="ExternalInput").ap()
    mb_in = nc.dram_tensor("mb_in", [lk], F32, kind="ExternalInput").ap()
    wq_in = nc.dram_tensor("wq_in", [D, D], F32, kind="ExternalInput").ap()
    wk_in = nc.dram_tensor("wk_in", [D, D], F32, kind="ExternalInput").ap()
    bq_in = nc.dram_tensor("bq_in", [D], F32, kind="ExternalInput").ap()
    bk_in = nc.dram_tensor("bk_in", [D], F32, kind="ExternalInput").ap()
    out = nc.dram_tensor("out", [L, D], F32, kind="ExternalOutput").ap()

    with tile.TileContext(nc) as tc:
        with (
            tc.tile_pool(name="consts", bufs=1) as consts,
            tc.tile_pool(name="stage", bufs=1) as stage,
            tc.tile_pool(name="acts", bufs=1) as acts,
            tc.tile_pool(name="pt", bufs=t_lk + 3) as ptp,
            tc.tile_pool(name="eplg", bufs=4) as eplg,
            tc.tile_pool(name="pp_ps", bufs=2, space="PSUM") as pp_ps,
            tc.tile_pool(name="s_ps", bufs=3, space="PSUM") as s_ps,
        ):
            # ---- constants; gather indices first (gathers gate the k path)
            idx_sb = consts.tile([P, t_lk], mybir.dt.int32)
            nc.sync.dma_start(out=idx_sb, in_=idx_in.rearrange("(t p) -> p t", p=P))
            ident = consts.tile([P, P], mm_dt)
            nc.scalar.dma_start(out=ident, in_=id_in)

            # ---- load query; gather key/value rows -------------------------
            q_stage = stage.tile([P, t_lq, D], mm_dt)
            q_view = q_in.rearrange("(t p) d -> p t d", p=P).bitcast(mm_dt)
            for c in range(0, t_lq, 4):
                nc.sync.dma_start(out=q_stage[:, c:c + 4, :],
                                  in_=q_view[:, c:c + 4, :])

            wq_sb = consts.tile([P, 2, D], mm_dt)
            nc.scalar.dma_start(out=wq_sb,
                              in_=wq_in.rearrange("(a p) d -> p a d", p=P).bitcast(mm_dt))
            wk_sb = consts.tile([P, 2, D], mm_dt)
            nc.scalar.dma_start(out=wk_sb,
                              in_=wk_in.rearrange("(a p) d -> p a d", p=P).bitcast(mm_dt))
            bq_sb = consts.tile([P, 2], F32)
            nc.scalar.dma_start(out=bq_sb, in_=bq_in.rearrange("(h p) -> p h", p=P))
            bk_sb = consts.tile([P, 2], F32)
            nc.scalar.dma_start(out=bk_sb, in_=bk_in.rearrange("(h p) -> p h", p=P))
            mb_sb = consts.tile([P, t_lk], F32)
            nc.scalar.dma_start(out=mb_sb, in_=mb_in.rearrange("(t p) -> p t", p=P))

            k_stage = stage.tile([P, t_lk, D], mm_dt)
            v_stage = stage.tile([P, t_lk, D], F32)
            for srcd, dst in ((k_in.bitcast(mm_dt), k_stage),
                              (v_in, v_stage)):
                for t in range(t_lk):
                    nc.gpsimd.indirect_dma_start(
                        out=dst[:, t, :],
                        out_offset=None,
                        in_=srcd,
                        in_offset=bass.IndirectOffsetOnAxis(
                            ap=idx_sb[:, t:t + 1], axis=0),
                    )

            # ---- PE warm-up: HAM un-throttles (1.2 -> 2.4 GHz) only after
            # ~3.4us of sustained activity, and re-throttles after ~3.4us
            # idle. Tiny matmuls on a memset scratch (no DMA dependency)
            # bridge the DMA front so the real work starts at full clock.
            warm_sb = consts.tile([P, 16], mm_dt)
            nc.vector.memset(warm_sb.bitcast(F32), 1.0)
            for _ in range(32):
                wp = pp_ps.tile([16, 16], F32, tag="pp")
                nc.tensor.matmul(wp, lhsT=warm_sb, rhs=warm_sb,
                                 start=True, stop=True)

            # ---- transpose + project, interleaved per 512-row chunk so the
            # PE follows data arrival (q first, then gathered k) ------------
            q_tr = acts.tile([P, 2, L], mm_dt)
            k_tr = acts.tile([P, 2, lk], mm_dt)
            q_t = acts.tile([P, 2, L], mm_dt)
            k_t = acts.tile([P, 2, lk], mm_dt)
            q_path = (q_stage, q_tr, wq_sb, bq_sb, q_t, t_lq, L)
            k_path = (k_stage, k_tr, wk_sb, bk_sb, k_t, t_lk, lk)

            def emit_chunk(path, c0):
                srcd, dst, w_sb, b_sb, x_t, nt, ln = path
                cw = min(512, ln - c0)
                for t in range(c0 // P, min(nt, (c0 + 512) // P)):
                    for h in range(2):
                        tp = s_ps.tile([P, P], mm_dt, tag="sp")
                        nc.tensor.transpose(
                            out=tp,
                            in_=srcd[:, t, h * P:(h + 1) * P],
                            identity=ident)
                        nc.vector.tensor_copy(
                            out=dst[:, h, t * P:(t + 1) * P], in_=tp)
                for h in range(2):
                    pp = pp_ps.tile([P, 512], F32, tag="pp")
                    for a in range(2):
                        nc.tensor.matmul(
                            pp[:, :cw],
                            lhsT=w_sb[:, a, h * P:(h + 1) * P],
                            rhs=dst[:, a, c0:c0 + cw],
                            start=(a == 0), stop=(a == 1))
                    nc.vector.tensor_scalar_add(
                        out=x_t[:, h, c0:c0 + cw], in0=pp[:, :cw],
                        scalar1=b_sb[:, h:h + 1])

            # interleave q/k chunks by expected data arrival: q is on-chip
            # early, gathered k tiles trickle in from the serial Q7 gathers
            q_chunks = [(q_path, c0) for c0 in range(0, L, 512)]
            k_chunks = [(k_path, c0) for c0 in range(0, lk, 512)]
            order = []
            qi = ki = 0
            for slot in range(len(q_chunks) + len(k_chunks)):
                take_q = qi < len(q_chunks) and (slot < 2 or slot % 2 == 1
                                                 or ki >= len(k_chunks))
                if take_q:
                    order.append(q_chunks[qi]); qi += 1
                elif ki < len(k_chunks):
                    order.append(k_chunks[ki]); ki += 1
            for path, c0 in order:
                emit_chunk(path, c0)

            # V with a ones-column appended: PV accumulates the softmax
            # denominator into column 256. Width D+2 keeps the free size
            # even and offsets 8B-aligned (fp32r matmul ISA restriction).
            # Built here (not earlier): only needed once PV starts.
            v_ext = acts.tile([P, t_lk, D + 2], mm_dt)
            for t0 in range(0, t_lk, 3):
                t1 = min(t0 + 3, t_lk)
                nc.gpsimd.tensor_copy(out=v_ext[:, t0:t1, 0:D],
                                      in_=v_stage[:, t0:t1, :])
            # memset can't encode an fp32r value type; 1.0f is bit-identical
            nc.gpsimd.memset(v_ext[:, :, D:D + 2].bitcast(F32), 1.0)

            # ---- attention over lq blocks ----------------------------------
            for blk in range(n_blk):
                b0 = blk * lq_blk
                pts = []
                for t in range(t_lk):
                    sp = s_ps.tile([P, lq_blk], F32, tag="sp")
                    for h in range(2):
                        for c0 in range(0, lq_blk, 512):
                            nc.tensor.matmul(
                                sp[:, c0:c0 + 512],
                                lhsT=k_t[:, h, t * P:(t + 1) * P],
                                rhs=q_t[:, h, b0 + c0:b0 + c0 + 512],
                                start=(h == 0), stop=(h == 1))
                    pt = ptp.tile([P, lq_blk], mm_dt, tag="pt")
                    nc.scalar.activation(out=pt, in_=sp,
                                         func=mybir.ActivationFunctionType.Exp,
                                         bias=mb_sb[:, t:t + 1], scale=1.0)
                    pts.append(pt)

                for qt in range(lq_blk // P):
                    op = pp_ps.tile([P, D + 2], F32, tag="pp")
                    for t in range(t_lk):
                        nc.tensor.matmul(
                            op,
                            lhsT=pts[t][:, qt * P:(qt + 1) * P],
                            rhs=v_ext[:, t, :],
                            start=(t == 0), stop=(t == t_lk - 1))
                    rcp = eplg.tile([P, 1], F32, tag="rcp")
                    nc.vector.reciprocal(out=rcp, in_=op[:, D:D + 1])
                    o_sb = eplg.tile([P, D], F32, tag="osb")
                    nc.vector.tensor_scalar_mul(out=o_sb, in0=op[:, 0:D],
                                                scalar1=rcp)
                    r0 = b0 + qt * P
                    nc.scalar.dma_start(out=out[r0:r0 + P, :], in_=o_sb)

    return nc


_PROG_CACHE = {}


def _get_program(lk, mm_dt=F32R):
    key = (lk, str(mm_dt))
    if key not in _PROG_CACHE:
        nc = _build(lk, mm_dt)
        # populate .instr bytes for InstISA subclasses (the library reload);
        # raw Bass skips the Bacc pass that does this
        mybir.codegen_inst_isa_subclasses(nc)
        _split_excess_waits(nc)  # only needed for walrus codegen (HW path)
        _PROG_CACHE[key] = nc
    return _PROG_CACHE[key]


def _prep_inputs(query, key, value, mask, Wq_w, Wq_b, Wk_w, Wk_b):
    """Host-side prep: fold 1/sqrt(D) into Wq, transpose weights, build
    per-batch gather indices + additive mask bias for padding."""
    scale = 1.0 / math.sqrt(D)
    wq_t = np.ascontiguousarray((Wq_w * scale).T, dtype=np.float32)
    wk_t = np.ascontiguousarray(Wk_w.T, dtype=np.float32)
    bq = (Wq_b * scale).astype(np.float32)
    bk = Wk_b.astype(np.float32)

    idxs = [np.nonzero(mask[b])[0] for b in range(B)]
    counts = [len(ix) for ix in idxs]
    lk = max(P, -(-max(counts) // P) * P)  # round up to multiple of 128

    in_maps = []
    for b in range(B):
        n = counts[b]
        idx = np.zeros(lk, dtype=np.int32)
        idx[:n] = idxs[b]
        mb = np.zeros(lk, dtype=np.float32)
        mb[n:] = NEG
        in_maps.append({
            "q_in": np.ascontiguousarray(query[b], dtype=np.float32),
            "k_in": np.ascontiguousarray(key[b], dtype=np.float32),
            "v_in": np.ascontiguousarray(value[b], dtype=np.float32),
            "idx_in": idx,
            "id_in": np.eye(P, dtype=np.float32),
            "mb_in": mb,
            "wq_in": wq_t, "wk_in": wk_t, "bq_in": bq, "bk_in": bk,
        })
    return in_maps, lk, counts


def _reference_batch_np(query, key, value, mask, Wq_w, Wq_b, Wk_w, Wk_b):
    """Exact numpy replica of the reference for degenerate batches
    (a batch whose mask is all zeros -> uniform softmax)."""
    q = query @ Wq_w.T + Wq_b
    k = key @ Wk_w.T + Wk_b
    s = (q @ k.T) / math.sqrt(D)
    m = mask.astype(s.dtype)[None, :]
    s = np.where(m == 0, np.float32(NEG), s * m)
    s = s - s.max(-1, keepdims=True)
    e = np.exp(s)
    attn = e / e.sum(-1, keepdims=True)
    return (attn @ value).astype(np.float32)


def _run(inputs, mm_dt=F32R, trace=False):
    query = np.asarray(inputs["query"], dtype=np.float32)
    key = np.asarray(inputs["key"], dtype=np.float32)
    value = np.asarray(inputs["value"], dtype=np.float32)
    mask = np.asarray(inputs["mask"])
    Wq_w = np.asarray(inputs["Wq_w"], dtype=np.float32)
    Wq_b = np.asarray(inputs["Wq_b"], dtype=np.float32)
    Wk_w = np.asarray(inputs["Wk_w"], dtype=np.float32)
    Wk_b = np.asarray(inputs["Wk_b"], dtype=np.float32)

    in_maps, lk, counts = _prep_inputs(query, key, value, mask,
                                       Wq_w, Wq_b, Wk_w, Wk_b)
    nc = _get_program(lk, mm_dt)
    res = run_bass_kernel_spmd(nc, in_maps, core_ids=list(range(B)),
                               trace=trace)
    out = np.stack([res.results[b]["out"] for b in range(B)])

    for b in range(B):
        if counts[b] == 0:  # degenerate: softmax over all -1e9 is uniform
            out[b] = _reference_batch_np(query[b], key[b], value[b], mask[b],
                                         Wq_w, Wq_b, Wk_w, Wk_b)
    return out, res


def kernel(**inputs) -> np.ndarray:
    out, _ = _run(inputs)
    return out
